# revision 20
# baseline (speedup 1.0000x reference)
"""Locally-connected layer (unshared 3x3 conv, torch-unfold semantics) on 8 trn2 cores.

out[b,o,y,x] = sum_{c,i,j} weight[o, c*9+i*3+j, y*32+x] * xpad[b, c, y+i, x+j] + bias[o, l]

Sharding: spatial over L - core r owns image rows [4r, 4r+4) (128 pixels).

Design history (measured on HW):
  v1 385 us: fp32, strided weight DMA -> 128 B packets, 10% MBU.
  v2/v3 63/69 us: bf16 + host relayout of weights into the SBUF stream
      layout. K=128 matmuls leave LDWEIGHTS serialized with the stream.
  v4 crash: psum group mixing base-0/base-64 matmuls (TRN2 erratum).
  v6 72 us: column-parity row groups; but both row groups share the single
      moving-operand XBUS, so streams serialize anyway.
  v7-v10 61-58 us: 64x64 array tiling works (matmul start-to-start ~26-34
      ns, 4-way); losses are all DMA orchestration: stores queued behind
      weights on a shared FIFO ring back-pressured psum evacuation (fixed
      by separate store ring); two-ring input splits halve per-ring
      delivery (fixed by single input ring); DMA completion semaphores
      fire ~2-6 us after the data (lag scales with DMA size).
  v11: COLUMN TILING ONLY (tiles T0=(0,0) and T2=(0,64)): two output
      columns in flight, both reading stationaries+weights from SBUF
      partitions 0:64 -> the x-slab upper-half duplicate disappears
      (input bytes 12.8 -> 11.1 MB; the input wall at ~425 GB/s aggregate
      dominates total time). Column tiling allocates separate XBUSes, so
      the two matmul streams and their LDWEIGHTS still overlap. Weight
      DMAs are split per pair-of-columns (0.59 MB) to shrink the
      completion-semaphore lag. All inputs ride the sync HWDGE ring in
      consumption order (one ring's packets use all 16 SDMA engines);
      stores ride the otherwise-empty scalar ring.

Per output column: 18 K=64 matmuls (N = vi*64 <= 192) into one psum bank,
homogeneous accumulation group per tile (erratum-safe). Even column of a
pair -> T0 -> psum partitions 0:64, odd -> T2 -> psum 64:128. PSUM
evacuation: VectorE (lower) / ScalarE (upper), into a parity-split staging
tile stored per 2 pairs.

Per-core HBM: w 9.44 MB + x 1.67 MB in, out 1.05 MB.
"""

import numpy as np

B, C, O, H, W, KS = 64, 64, 64, 32, 32, 3
L = H * W
NCORES = 8
RPC = H // NCORES            # image rows per core = 4
SLABR = RPC + 2              # slab rows per core (with halo) = 6
XS = W + 2                   # padded slab width = 34
NPAIR = W // 2               # 16 column pairs

# output rows y served by slab row rp: y = rp - i, i in 0..2, clipped
YS = [max(0, rp - 2) for rp in range(SLABR)]
VI = [min(RPC - 1, rp) - max(0, rp - 2) + 1 for rp in range(SLABR)]

# per-column block list: (rp, j, ya, n, off); off = cumulative stream column
BLOCKS = []
_off = 0
for _rp in range(SLABR):
    for _j in range(KS):
        _n = VI[_rp] * O
        BLOCKS.append((_rp, _j, YS[_rp] * O, _n, _off))
        _off += _n
CCOLS = _off                 # 2304 weight stream cols per output column
assert CCOLS == 2304

_CACHE = {}


def _build_nc():
    import concourse.bass as bass
    import concourse.bacc as bacc
    import concourse.tile as tile
    from concourse import mybir

    f32 = mybir.dt.float32
    bf16 = mybir.dt.bfloat16
    nc = bacc.Bacc(
        "TRN2", target_bir_lowering=False, debug=False, num_devices=NCORES
    )
    # x slab: [c, xs, rp, b]
    x_d = nc.dram_tensor("xf", [64, XS, SLABR, B], bf16, kind="ExternalInput")
    # weight stream: [c, pair, i, ccol]; col = 2*pair + i
    wg_d = nc.dram_tensor("wg", [64, NPAIR, 2, CCOLS], bf16,
                          kind="ExternalInput")
    # out, parity-split: [parity, b, pair, (y, o)]
    o_d = nc.dram_tensor("out", [2, B, NPAIR, RPC * O], bf16,
                         kind="ExternalOutput")

    with tile.TileContext(nc) as tc:
        with (
            tc.tile_pool(name="xp", bufs=1) as xpool,
            tc.tile_pool(name="wg", bufs=1) as wgpool,
            tc.tile_pool(name="ot", bufs=4) as opool,
            tc.tile_pool(name="ps", bufs=4, space=bass.MemorySpace.PSUM) as pspool,
        ):
            # all weight tiles resident (74 KB/partition on 0:64): every
            # input DMA is emitted up-front on the sync ring in consumption
            # order - descriptors queue ahead, the 16 SDMA engines never
            # starve, and arrival order exactly matches consumption order.
            xf = xpool.tile([64, XS, SLABR, B], bf16)
            wgs = [
                wgpool.tile([64, 1, 2, CCOLS], bf16, name=f"wgk{p}", tag=f"w{p}")
                for p in range(NPAIR)
            ]
            SPLIT = 1152     # block boundary: k=0..8 below, k=9..17 above
            nc.sync.dma_start(xf[:, 0:6], x_d[:, 0:6])
            nc.sync.dma_start(wgs[0][:, :, :, 0:SPLIT],
                              wg_d[:, 0:1, :, 0:SPLIT])
            nc.sync.dma_start(wgs[0][:, :, :, SPLIT:], wg_d[:, 0:1, :, SPLIT:])
            for p in range(1, NPAIR):
                if p == 2:
                    nc.sync.dma_start(xf[:, 6:18], x_d[:, 6:18])
                elif p == 8:
                    nc.sync.dma_start(xf[:, 18:34], x_d[:, 18:34])
                nc.sync.dma_start(wgs[p][:], wg_d[:, p : p + 1])

            ot = None
            for p in range(NPAIR):
                wgk = wgs[p]
                if p % 2 == 0:
                    ot = opool.tile([128, 2, RPC * O], bf16,
                                    name=f"ot{p}", tag="ot")
                oi = p % 2
                # even column -> T0 (psum 0:64), odd -> T2 (psum 64:128);
                # interleaved issue, independent tiles -> streams overlap
                ps0 = pspool.tile([128, RPC * O], f32, name="psA", tag="psA")
                ps1 = pspool.tile([128, RPC * O], f32, name="psB", tag="psB")
                last = len(BLOCKS) - 1
                for k, (rp, j, ya, n, off) in enumerate(BLOCKS):
                    nc.tensor.matmul(
                        ps0[0:64, ya : ya + n],
                        xf[:, 2 * p + j, rp, :],
                        wgk[:, 0, 0, off : off + n],
                        start=(k == 0), stop=(k == last),
                    )
                    nc.tensor.matmul(
                        ps1[64:128, ya : ya + n],
                        xf[:, 2 * p + 1 + j, rp, :],
                        wgk[:, 0, 1, off : off + n],
                        start=(k == 0), stop=(k == last),
                    )
                # evacuate: VectorE lower half, ScalarE upper half
                nc.vector.tensor_copy(ot[0:64, oi, :], ps0[0:64, :])
                nc.scalar.copy(ot[64:128, oi, :], ps1[64:128, :])
                # stores ride the otherwise-empty scalar HWDGE ring
                if p % 2 == 1:
                    nc.scalar.dma_start(o_d[0, :, p - 1 : p + 1], ot[0:64])
                    nc.scalar.dma_start(o_d[1, :, p - 1 : p + 1], ot[64:128])
    nc.compile()
    return nc


def _get_nc():
    if "nc" not in _CACHE:
        _CACHE["nc"] = _build_nc()
    return _CACHE["nc"]


def _shard_inputs(x, weight):
    from concourse import mybir

    bf16 = mybir.dt.np(mybir.dt.bfloat16)

    xpad = np.pad(x, ((0, 0), (0, 0), (1, 1), (1, 1)))  # (B, C, 34, 34)
    XF = np.zeros((NCORES, 64, XS, SLABR, B), np.float32)
    base = xpad.transpose(1, 3, 2, 0)  # (c, col, row, b)
    for rp in range(SLABR):
        # slab row rp of core r is padded row 4r+rp (8 cores)
        XF[:, :, :, rp, :] = (
            base[:, :, rp : rp + 4 * NCORES : 4, :].transpose(2, 0, 1, 3)
        )

    # weight stream; w6[o, c, i, j, y_img, x]
    w6 = weight.reshape(O, C, KS, KS, H, W)
    WG = np.zeros((NCORES, 64, NPAIR, 2, CCOLS), np.float32)
    for rp, j, ya, n, off in BLOCKS:
        for yi in range(VI[rp]):
            y = YS[rp] + yi
            i = rp - y
            csl = slice(off + yi * O, off + (yi + 1) * O)
            # (O, C, R, W) -> (R, C, W, O); W -> (pair, parity)
            src = (
                w6[:, :, i, j, y::RPC, :].transpose(2, 1, 3, 0)
                .reshape(NCORES, C, NPAIR, 2, O)
            )
            WG[:, :, :, :, csl] = src
    XF = XF.astype(bf16)
    WG = WG.astype(bf16)
    return [{"xf": XF[r], "wg": WG[r]} for r in range(NCORES)]


def kernel(x, weight, bias, _trace=False, _trace_kwargs=None):
    from concourse.bass_utils import run_bass_kernel_spmd

    x = np.ascontiguousarray(np.asarray(x, dtype=np.float32))
    weight = np.asarray(weight, dtype=np.float32)
    bias = np.asarray(bias, dtype=np.float32)

    nc = _get_nc()
    in_maps = _shard_inputs(x, weight)
    res = run_bass_kernel_spmd(
        nc, in_maps, list(range(NCORES)),
        trace=_trace, **(_trace_kwargs or {}),
    )
    # per-core out [2, B, 16, 256]: col x = 2*pair + parity -> (B, O, y, x)
    rows = []
    for r in range(NCORES):
        o = np.asarray(res.results[r]["out"], dtype=np.float32)
        full = np.zeros((B, W, RPC, O), np.float32)
        full[:, 0::2] = o[0].reshape(B, NPAIR, RPC, O)
        full[:, 1::2] = o[1].reshape(B, NPAIR, RPC, O)
        rows.append(full.transpose(0, 3, 2, 1))  # (B, O, y, x)
    out = np.concatenate(rows, axis=2)  # (B, O, H, W)
    if np.any(bias):
        out = out + bias.reshape(1, O, H, W)
    if _trace:
        _CACHE["last_result"] = res
    return np.ascontiguousarray(out.astype(np.float32))
